# revision 25
# baseline (speedup 1.0000x reference)
"""ALiBi multi-head attention on 8 trn2 NeuronCores (Bass/Tile).

Sharding: head+batch parallel. 16 heads x 2 batches = 32 (b,h) pairs; each of
the 8 cores owns 2 heads x 2 batches = 4 pairs (tensor-parallel projections
over heads; out-projection partials summed on the host). Per batch: stream
xT, produce Q^T/K^T (head-dim on partitions) and V (PE-transposed), then
flash-style attention in the transposed layout — S^T[j,i] matmuls, ACT exp
with the ALiBi bias folded into the per-partition bias operand (the slope*i
term cancels in softmax_j and doubles as the stabilizer), ones-matmul
denominator, PV on unnormalized exp tiles, post-PV normalization split
DVE/GPSIMD, reciprocal via ACT ln->exp, out-projection interleaved per
i-tile. All matmuls fp32r (~1 cyc/col, 1.5e-4 matmul precision).
"""

import sys

sys.path.insert(0, "/opt/trn_rl_repo")

import numpy as np

import concourse.mybir as mybir
import concourse.tile as tile
from concourse import bacc
from concourse.bass_utils import run_bass_kernel_spmd
from concourse.masks import make_identity

B, T, C, H = 2, 2048, 2048, 16
HD = C // H
NCORES = 8
HPC = H // NCORES
TOK = B * T
KO = C // 128
NT = T // 512
F32 = mybir.dt.float32
F32R = mybir.dt.float32r
EXPF = mybir.ActivationFunctionType.Exp
MULT = mybir.AluOpType.mult
LAG = 3

_cached = {}


def _build():
    nc = bacc.Bacc(None, target_bir_lowering=False)

    xT_d = nc.dram_tensor("xT", [KO, 128, TOK], F32, kind="ExternalInput")
    wq_d = nc.dram_tensor("wqT", [KO, 128, 256], F32, kind="ExternalInput")
    wk_d = nc.dram_tensor("wkT", [KO, 128, 256], F32, kind="ExternalInput")
    wv_d = nc.dram_tensor("wvT", [KO, 128, 256], F32, kind="ExternalInput")
    wo_d = nc.dram_tensor("woT", [HPC, 128, C], F32, kind="ExternalInput")
    cb_d = nc.dram_tensor("colbias", [128, HPC * KO], F32, kind="ExternalInput")
    attn_d = nc.dram_tensor("attnT", [2 * HPC, T, T], F32, kind="ExternalOutput")
    out_d = nc.dram_tensor("outp", [TOK, C], F32, kind="ExternalOutput")

    def r(ap):
        return ap.bitcast(F32R)

    with tile.TileContext(nc) as tc:
        with tc.tile_pool(name="consts", bufs=1) as cp:
            tri = cp.tile([128, 128], F32)  # tri[p,q] = 1 if q >= p else 0
            nc.gpsimd.memset(tri[:], 1.0)
            nc.gpsimd.affine_select(
                out=tri[:], in_=tri[:], compare_op=mybir.AluOpType.is_ge,
                fill=0.0, base=0, pattern=[[1, 128]], channel_multiplier=-1,
            )
            ones_dram = nc.inline_tensor(np.ones((128, 128), np.float32), name="ones_c")
            ones = cp.tile([128, 128], F32R)
            nc.sync.dma_start(ones[:], ones_dram[:].bitcast(F32R))
            cb = cp.tile([128, HPC * KO], F32)
            nc.sync.dma_start(cb[:], cb_d[:])
            wo = cp.tile([128, HPC, C], F32R)
            nc.sync.dma_start(wo[:], r(wo_d[:].rearrange("h ki c -> ki h c")))

            for b in range(B):
                with tc.tile_pool(name=f"qkv{b}", bufs=1) as rp:
                    QT = rp.tile([128, HPC, T], F32, name="QT")
                    KT = rp.tile([128, HPC, T], F32, name="KT")
                    V = rp.tile([128, KO, 256], F32, name="V")

                    # ---- Phase A(b): projections for this batch ----
                    with (
                        tc.tile_pool(name=f"wp{b}", bufs=1) as wp,
                        tc.tile_pool(name=f"xt{b}", bufs=3) as xtp,
                        tc.tile_pool(name=f"vt{b}", bufs=2) as vtp,
                        tc.tile_pool(name=f"pa{b}", bufs=1, space="PSUM") as pa,
                        tc.tile_pool(name=f"tp{b}", bufs=2, space="PSUM") as tpp,
                    ):
                        wq = wp.tile([128, KO, 256], F32R, name="wq")
                        wk = wp.tile([128, KO, 256], F32R, name="wk")
                        wv = wp.tile([128, KO, 256], F32R, name="wv")
                        ident = wp.tile([128, 128], F32, name="ident")
                        make_identity(nc, ident[:])
                        nc.sync.dma_start(wq[:], r(wq_d[:].rearrange("ko ki d -> ki ko d")))
                        nc.sync.dma_start(wk[:], r(wk_d[:].rearrange("ko ki d -> ki ko d")))
                        nc.sync.dma_start(wv[:], r(wv_d[:].rearrange("ko ki d -> ki ko d")))

                        for tloc in range(NT):
                            tg = b * T + tloc * 512  # global token offset
                            halves = []
                            for hh in range(2):
                                xh = xtp.tile([128, KO // 2, 512], F32R, tag="xt", name="xh")
                                nc.sync.dma_start(
                                    xh[:],
                                    r(
                                        xT_d[hh * 8 : hh * 8 + 8, :, tg : tg + 512]
                                        .rearrange("ko ki t -> ki ko t")
                                    ),
                                )
                                halves.append(xh)
                            ps = {}
                            for j, nm in enumerate(("q0", "q1", "k0", "k1", "v0", "v1")):
                                ps[nm] = pa.tile([128, 512], F32, tag=f"pp{j}", name=f"pp{j}")
                            for ko in range(KO):
                                xs = halves[ko // 8][:, ko % 8, :]
                                st = ko == 0
                                sp = ko == KO - 1
                                for h in range(HPC):
                                    ds = slice(h * 128, h * 128 + 128)
                                    nc.tensor.matmul(ps[f"q{h}"][:], wq[:, ko, ds], xs, start=st, stop=sp)
                                    nc.tensor.matmul(ps[f"k{h}"][:], wk[:, ko, ds], xs, start=st, stop=sp)
                                    nc.tensor.matmul(ps[f"v{h}"][:], wv[:, ko, ds], xs, start=st, stop=sp)
                            tsl = slice(tloc * 512, tloc * 512 + 512)
                            for h in range(HPC):
                                nc.scalar.copy(r(QT[:, h, tsl]), ps[f"q{h}"][:])
                                nc.scalar.copy(r(KT[:, h, tsl]), ps[f"k{h}"][:])
                                vt = vtp.tile([128, 512], F32, tag="vt", name="vt")
                                nc.scalar.copy(vt[:], ps[f"v{h}"][:])
                                for q in range(4):
                                    jc = tloc * 4 + q
                                    tp = tpp.tile([128, 128], F32, tag="tp", name="tp")
                                    nc.tensor.transpose(tp[:], vt[:, q * 128 : q * 128 + 128], ident[:])
                                    nc.vector.tensor_copy(
                                        out=r(V[:, jc, h * 128 : h * 128 + 128]),
                                        in_=tp[:],
                                    )

                    # ---- Phases B+C(b): attention + out-projection ----
                    # PV consumes UNNORMALIZED exp tiles so the PE stream
                    # (S^T, den, PV, out-proj) never waits on the softmax
                    # division; den/PV lag the S^T stream by LAG chunks;
                    # attn-tile normalization is post-PV (plain f32, split
                    # DVE/GPSIMD); attn_out^T normalized once per i-tile from
                    # PSUM; out-projection emitted per (b, i-tile).
                    attn_r = attn_d[:].rearrange("s (jc p) i -> s p jc i", p=128)
                    with (
                        tc.tile_pool(name=f"expb{b}", bufs=2) as expb,
                        tc.tile_pool(name=f"exps{b}", bufs=2) as exps,
                        tc.tile_pool(name=f"aot{b}", bufs=2) as aotp,
                        tc.tile_pool(name=f"sbb{b}", bufs=2) as sbb,
                        tc.tile_pool(name=f"stg{b}", bufs=3) as stp,
                        tc.tile_pool(name=f"osb{b}", bufs=2) as osb,
                        tc.tile_pool(name=f"pss{b}", bufs=3, space="PSUM") as pss,
                        tc.tile_pool(name=f"psd{b}", bufs=2, space="PSUM") as psd,
                        tc.tile_pool(name=f"psa{b}", bufs=1, space="PSUM") as psa,
                        tc.tile_pool(name=f"pso{b}", bufs=2, space="PSUM") as pso,
                    ):
                        for t in reversed(range(NT)):
                            J = 4 * (t + 1)
                            isl = slice(t * 512, t * 512 + 512)
                            aoT = aotp.tile([128, HPC, 512], F32, tag="aot", name="aoT")
                            for h in range(HPC):
                                slot = 2 * h + b
                                if t >= 2:
                                    eb = expb.tile([128, KO, 512], F32, tag="expb", name="eb")
                                else:
                                    eb = exps.tile([128, 8, 512], F32, tag="exps", name="eb")
                                den = psd.tile([128, 512], F32, tag="den", name="den")
                                ao = psa.tile([128, 512], F32, tag="ao", name="ao")

                                def rng(jc):
                                    return slice(max(128 * jc - 512 * t, 0), 512)

                                def den_pv(jc):
                                    nc.tensor.matmul(
                                        den[:, rng(jc)], ones[:], r(eb[:, jc, rng(jc)]),
                                        start=(jc == 0), stop=(jc == J - 1),
                                        skip_group_check=True,
                                    )
                                    nc.tensor.matmul(
                                        ao[:, rng(jc)],
                                        r(V[:, jc, h * 128 : h * 128 + 128]),
                                        r(eb[:, jc, rng(jc)]),
                                        start=(jc == 0), stop=(jc == J - 1),
                                        skip_group_check=True,
                                    )

                                for jc in range(J):
                                    sp = pss.tile([128, 512], F32, tag="s", name="sp")
                                    nc.tensor.matmul(
                                        sp[:],
                                        r(KT[:, h, jc * 128 : jc * 128 + 128]),
                                        r(QT[:, h, isl]),
                                        start=True, stop=True,
                                    )
                                    bias = cb[:, h * KO + jc : h * KO + jc + 1]
                                    off = 128 * jc - 512 * t
                                    nc.scalar.activation(
                                        r(eb[:, jc, rng(jc)]), sp[:, rng(jc)], EXPF,
                                        bias=bias,
                                    )
                                    if off >= 0:  # causal triangle on diagonal
                                        nc.vector.tensor_mul(
                                            r(eb[:, jc, off : off + 128]),
                                            eb[:, jc, off : off + 128],
                                            tri[:],
                                        )
                                    if jc >= LAG:
                                        den_pv(jc - LAG)
                                for jc in range(max(J - LAG, 0), J):
                                    den_pv(jc)
                                # reciprocal via ACT ln -> exp(-x)
                                rcp = sbb.tile([128, 512], F32, tag="rcp", name="rcp")
                                nc.vector.reciprocal(rcp[:], den[:])
                                nc.vector.tensor_tensor(
                                    r(aoT[:, h, :]), ao[:], rcp[:], MULT
                                )
                                # normalize into staging tiles (plain f32 —
                                # GPSIMD-writable, never aliased with f32r
                                # matmul reads) and DMA each to attn output
                                for jc in range(J):
                                    eng = nc.vector if jc % 2 == 0 else nc.gpsimd
                                    st = stp.tile([128, 512], F32, tag="st", name="st")
                                    g = rng(jc)
                                    eng.tensor_mul(st[:, g], eb[:, jc, g], rcp[:, g])
                                    off = max(128 * jc - 512 * t, 0)
                                    nc.sync.dma_start(
                                        attn_r[slot, :, jc, t * 512 + off : (t + 1) * 512],
                                        st[:, g],
                                    )
                            # out-projection for this batch's i-chunks
                            for ic in range(4 * t, 4 * t + 4):
                                ob = osb.tile([128, C], F32, tag="ob", name="ob")
                                csl = slice(ic * 128, ic * 128 + 128)
                                for ct in range(C // 512):
                                    op = pso.tile([128, 512], F32, tag="o", name="op")
                                    for h in range(HPC):
                                        nc.tensor.matmul(
                                            op[:],
                                            r(aoT[:, h, (ic - 4 * t) * 128 : (ic - 4 * t) * 128 + 128]),
                                            wo[:, h, ct * 512 : ct * 512 + 512],
                                            start=(h == 0), stop=(h == HPC - 1),
                                        )
                                    nc.vector.tensor_copy(
                                        out=ob[:, ct * 512 : ct * 512 + 512], in_=op[:]
                                    )
                                row0 = b * T + ic * 128
                                nc.sync.dma_start(out_d[row0 : row0 + 128, :], ob[:])

    nc.compile()
    return nc


def _get_nc():
    if "nc" not in _cached:
        _cached["nc"] = _build()
    return _cached["nc"]


def kernel(x, Wq, Wk, Wv, Wo, bo):
    x = np.asarray(x, np.float32)
    Wq = np.asarray(Wq, np.float32)
    Wk = np.asarray(Wk, np.float32)
    Wv = np.asarray(Wv, np.float32)
    Wo = np.asarray(Wo, np.float32)
    bo = np.asarray(bo, np.float32)

    nc = _get_nc()
    scale = 1.0 / np.sqrt(HD)
    xTr = np.ascontiguousarray(x.reshape(TOK, C).T).reshape(KO, 128, TOK)
    slopes = np.array([1.0 / 2 ** (i + 1) for i in range(H)], np.float32)

    in_maps = []
    for c in range(NCORES):
        rows = slice(256 * c, 256 * c + 256)
        wqT = np.ascontiguousarray((Wq[rows] * scale).T).reshape(KO, 128, 256)
        wkT = np.ascontiguousarray(Wk[rows].T).reshape(KO, 128, 256)
        wvT = np.ascontiguousarray(Wv[rows].T).reshape(KO, 128, 256)
        woT = np.ascontiguousarray(Wo[:, rows].T).reshape(HPC, 128, C)
        p = np.arange(128, dtype=np.float32)[:, None]
        jcs = np.arange(KO, dtype=np.float32)[None, :]
        cbs = []
        for h in range(HPC):
            s = slopes[HPC * c + h]
            cbs.append(-s * (128.0 * jcs + p))  # [128, KO]
        colbias = np.concatenate(cbs, axis=1).astype(np.float32)
        in_maps.append(
            {"xT": xTr, "wqT": wqT, "wkT": wkT, "wvT": wvT, "woT": woT,
             "colbias": colbias}
        )

    res = run_bass_kernel_spmd(nc, in_maps, core_ids=list(range(NCORES)))

    out = np.zeros((TOK, C), np.float32)
    attn = np.empty((B, H, T, T), np.float32)
    for c in range(NCORES):
        out += res.results[c]["outp"]
        at = res.results[c]["attnT"]
        for slot in range(2 * HPC):
            b = slot % 2
            h = HPC * c + slot // 2
            attn[b, h] = at[slot].T
    out = (out + bo[None, :]).reshape(B, T, C)
    return out, attn


if __name__ == "__main__":
    rng = np.random.default_rng(0)
    x = rng.standard_normal((B, T, C)).astype(np.float32)
    Wq = (rng.standard_normal((C, C)) * 0.02).astype(np.float32)
    Wk = (rng.standard_normal((C, C)) * 0.02).astype(np.float32)
    Wv = (rng.standard_normal((C, C)) * 0.02).astype(np.float32)
    Wo = (rng.standard_normal((C, C)) * 0.02).astype(np.float32)
    bo = np.zeros((C,), np.float32)
    out, attn = kernel(x, Wq, Wk, Wv, Wo, bo)
    print("out", out.shape, out.dtype, "attn", attn.shape, attn.dtype)


# revision 27
# speedup vs baseline: 1.0662x; 1.0662x over previous
"""ALiBi multi-head attention on 8 trn2 NeuronCores (Bass/Tile).

Sharding: head+batch parallel. 16 heads x 2 batches = 32 (b,h) pairs; each of
the 8 cores owns 2 heads x 2 batches = 4 pairs (tensor-parallel projections
over heads; out-projection partials summed on the host). Per batch: stream
xT, produce Q^T/K^T (head-dim on partitions) and V (PE-transposed), then
flash-style attention in the transposed layout — S^T[j,i] matmuls, ACT exp
with the ALiBi bias folded into the per-partition bias operand (the slope*i
term cancels in softmax_j and doubles as the stabilizer), ones-matmul
denominator, PV on unnormalized exp tiles, post-PV normalization split
DVE/GPSIMD, reciprocal via ACT ln->exp, out-projection interleaved per
i-tile. All matmuls fp32r (~1 cyc/col, 1.5e-4 matmul precision).
"""

import sys

sys.path.insert(0, "/opt/trn_rl_repo")

import numpy as np

import concourse.mybir as mybir
import concourse.tile as tile
from concourse import bacc
from concourse.bass_utils import run_bass_kernel_spmd
from concourse.masks import make_identity

B, T, C, H = 2, 2048, 2048, 16
HD = C // H
NCORES = 8
HPC = H // NCORES
TOK = B * T
KO = C // 128
NT = T // 512
F32 = mybir.dt.float32
F32R = mybir.dt.float32r
EXPF = mybir.ActivationFunctionType.Exp
MULT = mybir.AluOpType.mult
LAG = 3

_cached = {}


def _build():
    nc = bacc.Bacc(None, target_bir_lowering=False)

    xT_d = nc.dram_tensor("xT", [KO, 128, TOK], F32, kind="ExternalInput")
    wq_d = nc.dram_tensor("wqT", [KO, 128, 256], F32, kind="ExternalInput")
    wk_d = nc.dram_tensor("wkT", [KO, 128, 256], F32, kind="ExternalInput")
    wv_d = nc.dram_tensor("wvT", [KO, 128, 256], F32, kind="ExternalInput")
    wo_d = nc.dram_tensor("woT", [HPC, 128, C], F32, kind="ExternalInput")
    cb_d = nc.dram_tensor("colbias", [128, HPC * KO], F32, kind="ExternalInput")
    attn_d = nc.dram_tensor("attnT", [2 * HPC, T, T], F32, kind="ExternalOutput")
    out_d = nc.dram_tensor("outp", [TOK, C], F32, kind="ExternalOutput")

    def r(ap):
        return ap.bitcast(F32R)

    with tile.TileContext(nc) as tc:
        with tc.tile_pool(name="consts", bufs=1) as cp:
            tri = cp.tile([128, 128], F32)  # tri[p,q] = 1 if q >= p else 0
            nc.gpsimd.memset(tri[:], 1.0)
            nc.gpsimd.affine_select(
                out=tri[:], in_=tri[:], compare_op=mybir.AluOpType.is_ge,
                fill=0.0, base=0, pattern=[[1, 128]], channel_multiplier=-1,
            )
            ones_dram = nc.inline_tensor(np.ones((128, 128), np.float32), name="ones_c")
            ones = cp.tile([128, 128], F32R)
            nc.sync.dma_start(ones[:], ones_dram[:].bitcast(F32R))
            cb = cp.tile([128, HPC * KO], F32)
            nc.sync.dma_start(cb[:], cb_d[:])
            wo = cp.tile([128, HPC, C], F32R)
            nc.sync.dma_start(wo[:], r(wo_d[:].rearrange("h ki c -> ki h c")))

            for b in range(B):
                with tc.tile_pool(name=f"qkv{b}", bufs=1) as rp:
                    QT = rp.tile([128, HPC, T], F32, name="QT")
                    KT = rp.tile([128, HPC, T], F32, name="KT")
                    V = rp.tile([128, KO, 256], F32, name="V")

                    # ---- Phase A(b): projections for this batch ----
                    with (
                        tc.tile_pool(name=f"wp{b}", bufs=1) as wp,
                        tc.tile_pool(name=f"xt{b}", bufs=3) as xtp,
                        tc.tile_pool(name=f"vt{b}", bufs=2) as vtp,
                        tc.tile_pool(name=f"pa{b}", bufs=1, space="PSUM") as pa,
                        tc.tile_pool(name=f"tp{b}", bufs=2, space="PSUM") as tpp,
                    ):
                        wq = wp.tile([128, KO, 256], F32R, name="wq")
                        wk = wp.tile([128, KO, 256], F32R, name="wk")
                        wv = wp.tile([128, KO, 256], F32R, name="wv")
                        ident = wp.tile([128, 128], F32, name="ident")
                        make_identity(nc, ident[:])
                        nc.sync.dma_start(wq[:], r(wq_d[:].rearrange("ko ki d -> ki ko d")))
                        nc.sync.dma_start(wk[:], r(wk_d[:].rearrange("ko ki d -> ki ko d")))
                        nc.sync.dma_start(wv[:], r(wv_d[:].rearrange("ko ki d -> ki ko d")))

                        for tloc in range(NT):
                            tg = b * T + tloc * 512  # global token offset
                            halves = []
                            for hh in range(2):
                                xh = xtp.tile([128, KO // 2, 512], F32R, tag="xt", name="xh")
                                nc.sync.dma_start(
                                    xh[:],
                                    r(
                                        xT_d[hh * 8 : hh * 8 + 8, :, tg : tg + 512]
                                        .rearrange("ko ki t -> ki ko t")
                                    ),
                                )
                                halves.append(xh)
                            ps = {}
                            for j, nm in enumerate(("q0", "q1", "k0", "k1", "v0", "v1")):
                                ps[nm] = pa.tile([128, 512], F32, tag=f"pp{j}", name=f"pp{j}")
                            for ko in range(KO):
                                xs = halves[ko // 8][:, ko % 8, :]
                                st = ko == 0
                                sp = ko == KO - 1
                                for h in range(HPC):
                                    ds = slice(h * 128, h * 128 + 128)
                                    nc.tensor.matmul(ps[f"q{h}"][:], wq[:, ko, ds], xs, start=st, stop=sp)
                                    nc.tensor.matmul(ps[f"k{h}"][:], wk[:, ko, ds], xs, start=st, stop=sp)
                                    nc.tensor.matmul(ps[f"v{h}"][:], wv[:, ko, ds], xs, start=st, stop=sp)
                            tsl = slice(tloc * 512, tloc * 512 + 512)
                            for h in range(HPC):
                                nc.scalar.copy(r(QT[:, h, tsl]), ps[f"q{h}"][:])
                                nc.scalar.copy(r(KT[:, h, tsl]), ps[f"k{h}"][:])
                                vt = vtp.tile([128, 512], F32, tag="vt", name="vt")
                                nc.scalar.copy(vt[:], ps[f"v{h}"][:])
                                for q in range(4):
                                    jc = tloc * 4 + q
                                    tp = tpp.tile([128, 128], F32, tag="tp", name="tp")
                                    nc.tensor.transpose(tp[:], vt[:, q * 128 : q * 128 + 128], ident[:])
                                    nc.vector.tensor_copy(
                                        out=r(V[:, jc, h * 128 : h * 128 + 128]),
                                        in_=tp[:],
                                    )

                    # ---- Phases B+C(b): attention + out-projection ----
                    # PV consumes UNNORMALIZED exp tiles so the PE stream
                    # (S^T, den, PV, out-proj) never waits on the softmax
                    # division; den/PV lag the S^T stream by LAG chunks;
                    # attn-tile normalization is post-PV (plain f32, split
                    # DVE/GPSIMD); attn_out^T normalized once per i-tile from
                    # PSUM; out-projection emitted per (b, i-tile).
                    attn_r = attn_d[:].rearrange("s (jc p) i -> s p jc i", p=128)
                    with (
                        tc.tile_pool(name=f"expb{b}", bufs=2) as expb,
                        tc.tile_pool(name=f"exps{b}", bufs=2) as exps,
                        tc.tile_pool(name=f"aot{b}", bufs=2) as aotp,
                        tc.tile_pool(name=f"sbb{b}", bufs=2) as sbb,
                        tc.tile_pool(name=f"stg{b}", bufs=3) as stp,
                        tc.tile_pool(name=f"osb{b}", bufs=2) as osb,
                        tc.tile_pool(name=f"pss{b}", bufs=3, space="PSUM") as pss,
                        tc.tile_pool(name=f"psd{b}", bufs=2, space="PSUM") as psd,
                        tc.tile_pool(name=f"psa{b}", bufs=2, space="PSUM") as psa,
                        tc.tile_pool(name=f"pso{b}", bufs=1, space="PSUM") as pso,
                    ):
                        def emit_C(t, aoT):
                            # out-projection for i-chunks of i-tile t (lagged
                            # one t-step so its matmuls never wait the
                            # reciprocal/aoT-norm chain)
                            for ic in range(4 * t, 4 * t + 4):
                                ob = osb.tile([128, C], F32, tag="ob", name="ob")
                                for ct in range(C // 512):
                                    op = pso.tile([128, 512], F32, tag="o", name="op")
                                    for h in range(HPC):
                                        nc.tensor.matmul(
                                            op[:],
                                            r(aoT[:, h, (ic - 4 * t) * 128 : (ic - 4 * t) * 128 + 128]),
                                            wo[:, h, ct * 512 : ct * 512 + 512],
                                            start=(h == 0), stop=(h == HPC - 1),
                                        )
                                    nc.vector.tensor_copy(
                                        out=ob[:, ct * 512 : ct * 512 + 512], in_=op[:]
                                    )
                                row0 = b * T + ic * 128
                                nc.sync.dma_start(out_d[row0 : row0 + 128, :], ob[:])

                        pending_C = None
                        for t in reversed(range(NT)):
                            J = 4 * (t + 1)
                            isl = slice(t * 512, t * 512 + 512)
                            aoT = aotp.tile([128, HPC, 512], F32, tag="aot", name="aoT")
                            ebs, dens, aos = {}, {}, {}

                            def rng(jc):
                                return slice(max(128 * jc - 512 * t, 0), 512)

                            for h in range(HPC):
                                if t >= 2:
                                    eb = expb.tile([128, KO, 512], F32, tag="expb", name="eb")
                                else:
                                    eb = exps.tile([128, 8, 512], F32, tag="exps", name="eb")
                                den = psd.tile([128, 512], F32, tag="den", name="den")
                                ao = psa.tile([128, 512], F32, tag="ao", name="ao")
                                ebs[h], dens[h], aos[h] = eb, den, ao

                                def den_pv(jc):
                                    nc.tensor.matmul(
                                        den[:, rng(jc)], ones[:], r(eb[:, jc, rng(jc)]),
                                        start=(jc == 0), stop=(jc == J - 1),
                                        skip_group_check=True,
                                    )
                                    nc.tensor.matmul(
                                        ao[:, rng(jc)],
                                        r(V[:, jc, h * 128 : h * 128 + 128]),
                                        r(eb[:, jc, rng(jc)]),
                                        start=(jc == 0), stop=(jc == J - 1),
                                        skip_group_check=True,
                                    )

                                for jc in range(J):
                                    sp = pss.tile([128, 512], F32, tag="s", name="sp")
                                    nc.tensor.matmul(
                                        sp[:],
                                        r(KT[:, h, jc * 128 : jc * 128 + 128]),
                                        r(QT[:, h, isl]),
                                        start=True, stop=True,
                                    )
                                    bias = cb[:, h * KO + jc : h * KO + jc + 1]
                                    off = 128 * jc - 512 * t
                                    nc.scalar.activation(
                                        r(eb[:, jc, rng(jc)]), sp[:, rng(jc)], EXPF,
                                        bias=bias,
                                    )
                                    if off >= 0:  # causal triangle on diagonal
                                        nc.vector.tensor_mul(
                                            r(eb[:, jc, off : off + 128]),
                                            eb[:, jc, off : off + 128],
                                            tri[:],
                                        )
                                    if jc >= LAG:
                                        den_pv(jc - LAG)
                                for jc in range(max(J - LAG, 0), J):
                                    den_pv(jc)
                                # fast reciprocal (~18 bits, 1 DVE op) fires
                                # right after den completes; normalizes start
                                # during the next head's S^T stream
                                rcp = sbb.tile([128, 512], F32, tag="rcp", name="rcp")
                                nc.vector.reciprocal_approx_fast(out=rcp[:], in_=den[:])
                                nc.vector.tensor_tensor(
                                    r(aoT[:, h, :]), ao[:], rcp[:], MULT
                                )
                                slot = 2 * h + b
                                for jc in range(J):
                                    st = stp.tile([128, 512], F32, tag="st", name="st")
                                    g = rng(jc)
                                    nc.gpsimd.tensor_mul(
                                        st[:, g], eb[:, jc, g], rcp[:, g]
                                    )
                                    off = max(128 * jc - 512 * t, 0)
                                    nc.sync.dma_start(
                                        attn_r[slot, :, jc, t * 512 + off : (t + 1) * 512],
                                        st[:, g],
                                    )

                            # previous i-tile's out-projection fills PE while
                            # the softmax tails (DVE/GPSIMD) of this tile run
                            if pending_C is not None:
                                emit_C(*pending_C)
                            pending_C = (t, aoT)
                        emit_C(*pending_C)

    nc.compile()
    return nc


def _get_nc():
    if "nc" not in _cached:
        _cached["nc"] = _build()
    return _cached["nc"]


def kernel(x, Wq, Wk, Wv, Wo, bo):
    x = np.asarray(x, np.float32)
    Wq = np.asarray(Wq, np.float32)
    Wk = np.asarray(Wk, np.float32)
    Wv = np.asarray(Wv, np.float32)
    Wo = np.asarray(Wo, np.float32)
    bo = np.asarray(bo, np.float32)

    nc = _get_nc()
    scale = 1.0 / np.sqrt(HD)
    xTr = np.ascontiguousarray(x.reshape(TOK, C).T).reshape(KO, 128, TOK)
    slopes = np.array([1.0 / 2 ** (i + 1) for i in range(H)], np.float32)

    in_maps = []
    for c in range(NCORES):
        rows = slice(256 * c, 256 * c + 256)
        wqT = np.ascontiguousarray((Wq[rows] * scale).T).reshape(KO, 128, 256)
        wkT = np.ascontiguousarray(Wk[rows].T).reshape(KO, 128, 256)
        wvT = np.ascontiguousarray(Wv[rows].T).reshape(KO, 128, 256)
        woT = np.ascontiguousarray(Wo[:, rows].T).reshape(HPC, 128, C)
        p = np.arange(128, dtype=np.float32)[:, None]
        jcs = np.arange(KO, dtype=np.float32)[None, :]
        cbs = []
        for h in range(HPC):
            s = slopes[HPC * c + h]
            cbs.append(-s * (128.0 * jcs + p))  # [128, KO]
        colbias = np.concatenate(cbs, axis=1).astype(np.float32)
        in_maps.append(
            {"xT": xTr, "wqT": wqT, "wkT": wkT, "wvT": wvT, "woT": woT,
             "colbias": colbias}
        )

    res = run_bass_kernel_spmd(nc, in_maps, core_ids=list(range(NCORES)))

    out = np.zeros((TOK, C), np.float32)
    attn = np.empty((B, H, T, T), np.float32)
    for c in range(NCORES):
        out += res.results[c]["outp"]
        at = res.results[c]["attnT"]
        for slot in range(2 * HPC):
            b = slot % 2
            h = HPC * c + slot // 2
            attn[b, h] = at[slot].T
    out = (out + bo[None, :]).reshape(B, T, C)
    return out, attn


if __name__ == "__main__":
    rng = np.random.default_rng(0)
    x = rng.standard_normal((B, T, C)).astype(np.float32)
    Wq = (rng.standard_normal((C, C)) * 0.02).astype(np.float32)
    Wk = (rng.standard_normal((C, C)) * 0.02).astype(np.float32)
    Wv = (rng.standard_normal((C, C)) * 0.02).astype(np.float32)
    Wo = (rng.standard_normal((C, C)) * 0.02).astype(np.float32)
    bo = np.zeros((C,), np.float32)
    out, attn = kernel(x, Wq, Wk, Wv, Wo, bo)
    print("out", out.shape, out.dtype, "attn", attn.shape, attn.dtype)


# revision 28
# speedup vs baseline: 1.1661x; 1.0937x over previous
"""ALiBi multi-head attention on 8 trn2 NeuronCores (Bass/Tile).

Sharding: head+batch parallel. 16 heads x 2 batches = 32 (b,h) pairs; each of
the 8 cores owns 2 heads x 2 batches = 4 pairs (tensor-parallel projections
over heads; out-projection partials summed on the host). Per batch: stream
xT, produce Q^T/K^T (head-dim on partitions) and V (PE-transposed), then
flash-style attention in the transposed layout — S^T[j,i] matmuls, ACT exp
with the ALiBi bias folded into the per-partition bias operand (the slope*i
term cancels in softmax_j and doubles as the stabilizer), ones-matmul
denominator, PV on unnormalized exp tiles, post-PV normalization split
DVE/GPSIMD, reciprocal via ACT ln->exp, out-projection interleaved per
i-tile. All matmuls fp32r (~1 cyc/col, 1.5e-4 matmul precision).
"""

import sys

sys.path.insert(0, "/opt/trn_rl_repo")

import numpy as np

import concourse.mybir as mybir
import concourse.tile as tile
from concourse import bacc
from concourse.bass_utils import run_bass_kernel_spmd
from concourse.masks import make_identity

B, T, C, H = 2, 2048, 2048, 16
HD = C // H
NCORES = 8
HPC = H // NCORES
TOK = B * T
KO = C // 128
NT = T // 512
F32 = mybir.dt.float32
F32R = mybir.dt.float32r
EXPF = mybir.ActivationFunctionType.Exp
MULT = mybir.AluOpType.mult
LAG = 6

_cached = {}


def _build():
    nc = bacc.Bacc(None, target_bir_lowering=False)

    xT_d = nc.dram_tensor("xT", [KO, 128, TOK], F32, kind="ExternalInput")
    wq_d = nc.dram_tensor("wqT", [KO, 128, 256], F32, kind="ExternalInput")
    wk_d = nc.dram_tensor("wkT", [KO, 128, 256], F32, kind="ExternalInput")
    wv_d = nc.dram_tensor("wvT", [KO, 128, 256], F32, kind="ExternalInput")
    wo_d = nc.dram_tensor("woT", [HPC, 128, C], F32, kind="ExternalInput")
    cb_d = nc.dram_tensor("colbias", [128, HPC * KO], F32, kind="ExternalInput")
    attn_d = nc.dram_tensor("attnT", [2 * HPC, T, T], F32, kind="ExternalOutput")
    out_d = nc.dram_tensor("outp", [TOK, C], F32, kind="ExternalOutput")

    def r(ap):
        return ap.bitcast(F32R)

    with tile.TileContext(nc) as tc:
        with tc.tile_pool(name="consts", bufs=1) as cp:
            tri = cp.tile([128, 128], F32)  # tri[p,q] = 1 if q >= p else 0
            nc.gpsimd.memset(tri[:], 1.0)
            nc.gpsimd.affine_select(
                out=tri[:], in_=tri[:], compare_op=mybir.AluOpType.is_ge,
                fill=0.0, base=0, pattern=[[1, 128]], channel_multiplier=-1,
            )
            ones_dram = nc.inline_tensor(np.ones((128, 128), np.float32), name="ones_c")
            ones = cp.tile([128, 128], F32R)
            nc.sync.dma_start(ones[:], ones_dram[:].bitcast(F32R))
            cb = cp.tile([128, HPC * KO], F32)
            nc.sync.dma_start(cb[:], cb_d[:])
            wo = cp.tile([128, HPC, C], F32R)
            nc.sync.dma_start(wo[:], r(wo_d[:].rearrange("h ki c -> ki h c")))

            for b in range(B):
                with tc.tile_pool(name=f"qkv{b}", bufs=1) as rp:
                    QT = rp.tile([128, HPC, T], F32, name="QT")
                    KT = rp.tile([128, HPC, T], F32, name="KT")
                    V = rp.tile([128, KO, 256], F32, name="V")

                    # ---- Phase A(b): projections for this batch ----
                    with (
                        tc.tile_pool(name=f"wp{b}", bufs=1) as wp,
                        tc.tile_pool(name=f"xt{b}", bufs=3) as xtp,
                        tc.tile_pool(name=f"vt{b}", bufs=2) as vtp,
                        tc.tile_pool(name=f"pa{b}", bufs=1, space="PSUM") as pa,
                        tc.tile_pool(name=f"tp{b}", bufs=2, space="PSUM") as tpp,
                    ):
                        wq = wp.tile([128, KO, 256], F32R, name="wq")
                        wk = wp.tile([128, KO, 256], F32R, name="wk")
                        wv = wp.tile([128, KO, 256], F32R, name="wv")
                        ident = wp.tile([128, 128], F32, name="ident")
                        make_identity(nc, ident[:])
                        nc.sync.dma_start(wq[:], r(wq_d[:].rearrange("ko ki d -> ki ko d")))
                        nc.sync.dma_start(wk[:], r(wk_d[:].rearrange("ko ki d -> ki ko d")))
                        nc.sync.dma_start(wv[:], r(wv_d[:].rearrange("ko ki d -> ki ko d")))

                        for tloc in range(NT):
                            tg = b * T + tloc * 512  # global token offset
                            halves = []
                            for hh in range(2):
                                xh = xtp.tile([128, KO // 2, 512], F32R, tag="xt", name="xh")
                                nc.sync.dma_start(
                                    xh[:],
                                    r(
                                        xT_d[hh * 8 : hh * 8 + 8, :, tg : tg + 512]
                                        .rearrange("ko ki t -> ki ko t")
                                    ),
                                )
                                halves.append(xh)
                            ps = {}
                            for j, nm in enumerate(("q0", "q1", "k0", "k1", "v0", "v1")):
                                ps[nm] = pa.tile([128, 512], F32, tag=f"pp{j}", name=f"pp{j}")
                            for ko in range(KO):
                                xs = halves[ko // 8][:, ko % 8, :]
                                st = ko == 0
                                sp = ko == KO - 1
                                for h in range(HPC):
                                    ds = slice(h * 128, h * 128 + 128)
                                    nc.tensor.matmul(ps[f"q{h}"][:], wq[:, ko, ds], xs, start=st, stop=sp)
                                    nc.tensor.matmul(ps[f"k{h}"][:], wk[:, ko, ds], xs, start=st, stop=sp)
                                    nc.tensor.matmul(ps[f"v{h}"][:], wv[:, ko, ds], xs, start=st, stop=sp)
                            tsl = slice(tloc * 512, tloc * 512 + 512)
                            for h in range(HPC):
                                nc.scalar.copy(r(QT[:, h, tsl]), ps[f"q{h}"][:])
                                nc.scalar.copy(r(KT[:, h, tsl]), ps[f"k{h}"][:])
                                vt = vtp.tile([128, 512], F32, tag="vt", name="vt")
                                nc.scalar.copy(vt[:], ps[f"v{h}"][:])
                                for q in range(4):
                                    jc = tloc * 4 + q
                                    tp = tpp.tile([128, 128], F32, tag="tp", name="tp")
                                    nc.tensor.transpose(tp[:], vt[:, q * 128 : q * 128 + 128], ident[:])
                                    nc.vector.tensor_copy(
                                        out=r(V[:, jc, h * 128 : h * 128 + 128]),
                                        in_=tp[:],
                                    )

                    # ---- Phases B+C(b): attention + out-projection ----
                    # PV consumes UNNORMALIZED exp tiles so the PE stream
                    # (S^T, den, PV, out-proj) never waits on the softmax
                    # division; den/PV lag the S^T stream by LAG chunks;
                    # attn-tile normalization is post-PV (plain f32, split
                    # DVE/GPSIMD); attn_out^T normalized once per i-tile from
                    # PSUM; out-projection emitted per (b, i-tile).
                    attn_r = attn_d[:].rearrange("s (jc p) i -> s p jc i", p=128)
                    with (
                        tc.tile_pool(name=f"expb{b}", bufs=2) as expb,
                        tc.tile_pool(name=f"exps{b}", bufs=2) as exps,
                        tc.tile_pool(name=f"aot{b}", bufs=2) as aotp,
                        tc.tile_pool(name=f"sbb{b}", bufs=2) as sbb,
                        tc.tile_pool(name=f"stg{b}", bufs=3) as stp,
                        tc.tile_pool(name=f"osb{b}", bufs=2) as osb,
                        tc.tile_pool(name=f"pss{b}", bufs=2, space="PSUM") as pss,
                        tc.tile_pool(name=f"psd{b}", bufs=2, space="PSUM") as psd,
                        tc.tile_pool(name=f"psa{b}", bufs=2, space="PSUM") as psa,
                        tc.tile_pool(name=f"pso{b}", bufs=2, space="PSUM") as pso,
                    ):
                        def emit_C(t, aoT, ics):
                            # out-projection chunks of i-tile t (lagged one
                            # t-step, woven into the next tile's PE stream as
                            # ACT-independent filler work)
                            for ic in ics:
                                ob = osb.tile([128, C], F32, tag="ob", name="ob")
                                for ct in range(C // 512):
                                    op = pso.tile([128, 512], F32, tag="o", name="op")
                                    for h in range(HPC):
                                        nc.tensor.matmul(
                                            op[:],
                                            r(aoT[:, h, (ic - 4 * t) * 128 : (ic - 4 * t) * 128 + 128]),
                                            wo[:, h, ct * 512 : ct * 512 + 512],
                                            start=(h == 0), stop=(h == HPC - 1),
                                        )
                                    nc.vector.tensor_copy(
                                        out=ob[:, ct * 512 : ct * 512 + 512], in_=op[:]
                                    )
                                row0 = b * T + ic * 128
                                nc.sync.dma_start(out_d[row0 : row0 + 128, :], ob[:])

                        pending_C = None
                        for t in reversed(range(NT)):
                            J = 4 * (t + 1)
                            isl = slice(t * 512, t * 512 + 512)
                            aoT = aotp.tile([128, HPC, 512], F32, tag="aot", name="aoT")
                            ebs, dens, aos = {}, {}, {}

                            def rng(jc):
                                return slice(max(128 * jc - 512 * t, 0), 512)

                            for h in range(HPC):
                                if t >= 2:
                                    eb = expb.tile([128, KO, 512], F32, tag="expb", name="eb")
                                else:
                                    eb = exps.tile([128, 8, 512], F32, tag="exps", name="eb")
                                den = psd.tile([128, 512], F32, tag="den", name="den")
                                ao = psa.tile([128, 512], F32, tag="ao", name="ao")
                                ebs[h], dens[h], aos[h] = eb, den, ao

                                def den_pv(jc):
                                    nc.tensor.matmul(
                                        den[:, rng(jc)], ones[:], r(eb[:, jc, rng(jc)]),
                                        start=(jc == 0), stop=(jc == J - 1),
                                        skip_group_check=True,
                                    )
                                    nc.tensor.matmul(
                                        ao[:, rng(jc)],
                                        r(V[:, jc, h * 128 : h * 128 + 128]),
                                        r(eb[:, jc, rng(jc)]),
                                        start=(jc == 0), stop=(jc == J - 1),
                                        skip_group_check=True,
                                    )

                                for jc in range(J):
                                    sp = pss.tile([128, 512], F32, tag="s", name="sp")
                                    nc.tensor.matmul(
                                        sp[:],
                                        r(KT[:, h, jc * 128 : jc * 128 + 128]),
                                        r(QT[:, h, isl]),
                                        start=True, stop=True,
                                    )
                                    bias = cb[:, h * KO + jc : h * KO + jc + 1]
                                    off = 128 * jc - 512 * t
                                    nc.scalar.activation(
                                        r(eb[:, jc, rng(jc)]), sp[:, rng(jc)], EXPF,
                                        bias=bias,
                                    )
                                    if off >= 0:  # causal triangle on diagonal
                                        nc.vector.tensor_mul(
                                            r(eb[:, jc, off : off + 128]),
                                            eb[:, jc, off : off + 128],
                                            tri[:],
                                        )
                                    if jc >= LAG:
                                        den_pv(jc - LAG)
                                for jc in range(max(J - LAG, 0), J):
                                    den_pv(jc)
                                # weave half of the previous i-tile's
                                # out-projection after this head's stream
                                if pending_C is not None:
                                    tp_, aoTp = pending_C
                                    emit_C(tp_, aoTp,
                                           range(4 * tp_ + 2 * h, 4 * tp_ + 2 * h + 2))
                                # fast reciprocal (~18 bits, 1 DVE op) fires
                                # right after den completes; normalizes start
                                # during the next head's S^T stream
                                rcp = sbb.tile([128, 512], F32, tag="rcp", name="rcp")
                                nc.vector.reciprocal_approx_fast(out=rcp[:], in_=den[:])
                                nc.vector.tensor_tensor(
                                    r(aoT[:, h, :]), ao[:], rcp[:], MULT
                                )
                                slot = 2 * h + b
                                for jc in range(J):
                                    st = stp.tile([128, 512], F32, tag="st", name="st")
                                    g = rng(jc)
                                    nc.gpsimd.tensor_mul(
                                        st[:, g], eb[:, jc, g], rcp[:, g]
                                    )
                                    off = max(128 * jc - 512 * t, 0)
                                    nc.sync.dma_start(
                                        attn_r[slot, :, jc, t * 512 + off : (t + 1) * 512],
                                        st[:, g],
                                    )

                            pending_C = (t, aoT)
                        tp_, aoTp = pending_C
                        emit_C(tp_, aoTp, range(4 * tp_, 4 * tp_ + 4))

    nc.compile()
    return nc


def _get_nc():
    if "nc" not in _cached:
        _cached["nc"] = _build()
    return _cached["nc"]


def kernel(x, Wq, Wk, Wv, Wo, bo):
    x = np.asarray(x, np.float32)
    Wq = np.asarray(Wq, np.float32)
    Wk = np.asarray(Wk, np.float32)
    Wv = np.asarray(Wv, np.float32)
    Wo = np.asarray(Wo, np.float32)
    bo = np.asarray(bo, np.float32)

    nc = _get_nc()
    scale = 1.0 / np.sqrt(HD)
    xTr = np.ascontiguousarray(x.reshape(TOK, C).T).reshape(KO, 128, TOK)
    slopes = np.array([1.0 / 2 ** (i + 1) for i in range(H)], np.float32)

    in_maps = []
    for c in range(NCORES):
        rows = slice(256 * c, 256 * c + 256)
        wqT = np.ascontiguousarray((Wq[rows] * scale).T).reshape(KO, 128, 256)
        wkT = np.ascontiguousarray(Wk[rows].T).reshape(KO, 128, 256)
        wvT = np.ascontiguousarray(Wv[rows].T).reshape(KO, 128, 256)
        woT = np.ascontiguousarray(Wo[:, rows].T).reshape(HPC, 128, C)
        p = np.arange(128, dtype=np.float32)[:, None]
        jcs = np.arange(KO, dtype=np.float32)[None, :]
        cbs = []
        for h in range(HPC):
            s = slopes[HPC * c + h]
            cbs.append(-s * (128.0 * jcs + p))  # [128, KO]
        colbias = np.concatenate(cbs, axis=1).astype(np.float32)
        in_maps.append(
            {"xT": xTr, "wqT": wqT, "wkT": wkT, "wvT": wvT, "woT": woT,
             "colbias": colbias}
        )

    res = run_bass_kernel_spmd(nc, in_maps, core_ids=list(range(NCORES)))

    out = np.zeros((TOK, C), np.float32)
    attn = np.empty((B, H, T, T), np.float32)
    for c in range(NCORES):
        out += res.results[c]["outp"]
        at = res.results[c]["attnT"]
        for slot in range(2 * HPC):
            b = slot % 2
            h = HPC * c + slot // 2
            attn[b, h] = at[slot].T
    out = (out + bo[None, :]).reshape(B, T, C)
    return out, attn


if __name__ == "__main__":
    rng = np.random.default_rng(0)
    x = rng.standard_normal((B, T, C)).astype(np.float32)
    Wq = (rng.standard_normal((C, C)) * 0.02).astype(np.float32)
    Wk = (rng.standard_normal((C, C)) * 0.02).astype(np.float32)
    Wv = (rng.standard_normal((C, C)) * 0.02).astype(np.float32)
    Wo = (rng.standard_normal((C, C)) * 0.02).astype(np.float32)
    bo = np.zeros((C,), np.float32)
    out, attn = kernel(x, Wq, Wk, Wv, Wo, bo)
    print("out", out.shape, out.dtype, "attn", attn.shape, attn.dtype)


# revision 29
# speedup vs baseline: 1.1980x; 1.0274x over previous
"""ALiBi multi-head attention on 8 trn2 NeuronCores (Bass/Tile).

Sharding: head+batch parallel. 16 heads x 2 batches = 32 (b,h) pairs; each of
the 8 cores owns 2 heads x 2 batches = 4 pairs (tensor-parallel projections
over heads; out-projection partials summed on the host). Per batch: stream
xT, produce Q^T/K^T (head-dim on partitions) and V (PE-transposed), then
flash-style attention in the transposed layout — S^T[j,i] matmuls, ACT exp
with the ALiBi bias folded into the per-partition bias operand (the slope*i
term cancels in softmax_j and doubles as the stabilizer), ones-matmul
denominator, PV on unnormalized exp tiles, post-PV normalization split
DVE/GPSIMD, reciprocal via ACT ln->exp, out-projection interleaved per
i-tile. All matmuls fp32r (~1 cyc/col, 1.5e-4 matmul precision).
"""

import sys

sys.path.insert(0, "/opt/trn_rl_repo")

import numpy as np

import concourse.mybir as mybir
import concourse.tile as tile
from concourse import bacc
from concourse.bass_utils import run_bass_kernel_spmd
from concourse.masks import make_identity

B, T, C, H = 2, 2048, 2048, 16
HD = C // H
NCORES = 8
HPC = H // NCORES
TOK = B * T
KO = C // 128
NT = T // 512
F32 = mybir.dt.float32
F32R = mybir.dt.float32r
EXPF = mybir.ActivationFunctionType.Exp
MULT = mybir.AluOpType.mult
LAG = 6

_cached = {}


def _build():
    nc = bacc.Bacc(None, target_bir_lowering=False)

    xT_d = nc.dram_tensor("xT", [KO, 128, TOK], F32, kind="ExternalInput")
    wq_d = nc.dram_tensor("wqT", [KO, 128, 256], F32, kind="ExternalInput")
    wk_d = nc.dram_tensor("wkT", [KO, 128, 256], F32, kind="ExternalInput")
    wv_d = nc.dram_tensor("wvT", [KO, 128, 256], F32, kind="ExternalInput")
    wo_d = nc.dram_tensor("woT", [HPC, 128, C], F32, kind="ExternalInput")
    cb_d = nc.dram_tensor("colbias", [128, HPC * KO], F32, kind="ExternalInput")
    attn_d = nc.dram_tensor("attnT", [2 * HPC, T, T], F32, kind="ExternalOutput")
    out_d = nc.dram_tensor("outp", [TOK, C], F32, kind="ExternalOutput")

    def r(ap):
        return ap.bitcast(F32R)

    with tile.TileContext(nc) as tc:
        with tc.tile_pool(name="consts", bufs=1) as cp:
            tri = cp.tile([128, 128], F32)  # tri[p,q] = 1 if q >= p else 0
            nc.gpsimd.memset(tri[:], 1.0)
            nc.gpsimd.affine_select(
                out=tri[:], in_=tri[:], compare_op=mybir.AluOpType.is_ge,
                fill=0.0, base=0, pattern=[[1, 128]], channel_multiplier=-1,
            )
            ones_dram = nc.inline_tensor(np.ones((128, 128), np.float32), name="ones_c")
            ones = cp.tile([128, 128], F32R)
            nc.sync.dma_start(ones[:], ones_dram[:].bitcast(F32R))
            cb = cp.tile([128, HPC * KO], F32)
            nc.sync.dma_start(cb[:], cb_d[:])
            wo = cp.tile([128, HPC, C], F32R)
            nc.sync.dma_start(wo[:], r(wo_d[:].rearrange("h ki c -> ki h c")))

            for b in range(B):
                with tc.tile_pool(name=f"qkv{b}", bufs=1) as rp:
                    QT = rp.tile([128, HPC, T], F32, name="QT")
                    KT = rp.tile([128, HPC, T], F32, name="KT")
                    V = rp.tile([128, KO, 256], F32, name="V")

                    # ---- Phase A(b): projections for this batch ----
                    with (
                        tc.tile_pool(name=f"wp{b}", bufs=1) as wp,
                        tc.tile_pool(name=f"xt{b}", bufs=3) as xtp,
                        tc.tile_pool(name=f"vt{b}", bufs=2) as vtp,
                        tc.tile_pool(name=f"pa{b}", bufs=1, space="PSUM") as pa,
                        tc.tile_pool(name=f"tp{b}", bufs=2, space="PSUM") as tpp,
                    ):
                        wq = wp.tile([128, KO, 256], F32R, name="wq")
                        wk = wp.tile([128, KO, 256], F32R, name="wk")
                        wv = wp.tile([128, KO, 256], F32R, name="wv")
                        ident = wp.tile([128, 128], F32, name="ident")
                        make_identity(nc, ident[:])
                        for wt, wd in ((wq, wq_d), (wk, wk_d), (wv, wv_d)):
                            for hh in range(2):
                                ksl = slice(hh * 8, hh * 8 + 8)
                                nc.sync.dma_start(
                                    wt[:, ksl],
                                    r(wd[ksl].rearrange("ko ki d -> ki ko d")),
                                )

                        for tloc in range(NT):
                            tg = b * T + tloc * 512  # global token offset
                            halves = []
                            for hh in range(2):
                                xh = xtp.tile([128, KO // 2, 512], F32R, tag="xt", name="xh")
                                nc.sync.dma_start(
                                    xh[:],
                                    r(
                                        xT_d[hh * 8 : hh * 8 + 8, :, tg : tg + 512]
                                        .rearrange("ko ki t -> ki ko t")
                                    ),
                                )
                                halves.append(xh)
                            ps = {}
                            for j, nm in enumerate(("q0", "q1", "k0", "k1", "v0", "v1")):
                                ps[nm] = pa.tile([128, 512], F32, tag=f"pp{j}", name=f"pp{j}")
                            for ko in range(KO):
                                xs = halves[ko // 8][:, ko % 8, :]
                                st = ko == 0
                                sp = ko == KO - 1
                                for h in range(HPC):
                                    ds = slice(h * 128, h * 128 + 128)
                                    nc.tensor.matmul(ps[f"q{h}"][:], wq[:, ko, ds], xs, start=st, stop=sp)
                                    nc.tensor.matmul(ps[f"k{h}"][:], wk[:, ko, ds], xs, start=st, stop=sp)
                                    nc.tensor.matmul(ps[f"v{h}"][:], wv[:, ko, ds], xs, start=st, stop=sp)
                            tsl = slice(tloc * 512, tloc * 512 + 512)
                            for h in range(HPC):
                                nc.scalar.copy(r(QT[:, h, tsl]), ps[f"q{h}"][:])
                                nc.scalar.copy(r(KT[:, h, tsl]), ps[f"k{h}"][:])
                                vt = vtp.tile([128, 512], F32, tag="vt", name="vt")
                                nc.scalar.copy(vt[:], ps[f"v{h}"][:])
                                for q in range(4):
                                    jc = tloc * 4 + q
                                    tp = tpp.tile([128, 128], F32, tag="tp", name="tp")
                                    nc.tensor.transpose(tp[:], vt[:, q * 128 : q * 128 + 128], ident[:])
                                    nc.vector.tensor_copy(
                                        out=r(V[:, jc, h * 128 : h * 128 + 128]),
                                        in_=tp[:],
                                    )

                    # ---- Phases B+C(b): attention + out-projection ----
                    # PV consumes UNNORMALIZED exp tiles so the PE stream
                    # (S^T, den, PV, out-proj) never waits on the softmax
                    # division; den/PV lag the S^T stream by LAG chunks;
                    # attn-tile normalization is post-PV (plain f32, split
                    # DVE/GPSIMD); attn_out^T normalized once per i-tile from
                    # PSUM; out-projection emitted per (b, i-tile).
                    attn_r = attn_d[:].rearrange("s (jc p) i -> s p jc i", p=128)
                    with (
                        tc.tile_pool(name=f"expb{b}", bufs=2) as expb,
                        tc.tile_pool(name=f"exps{b}", bufs=2) as exps,
                        tc.tile_pool(name=f"aot{b}", bufs=2) as aotp,
                        tc.tile_pool(name=f"sbb{b}", bufs=2) as sbb,
                        tc.tile_pool(name=f"stg{b}", bufs=3) as stp,
                        tc.tile_pool(name=f"osb{b}", bufs=2) as osb,
                        tc.tile_pool(name=f"pss{b}", bufs=2, space="PSUM") as pss,
                        tc.tile_pool(name=f"psd{b}", bufs=2, space="PSUM") as psd,
                        tc.tile_pool(name=f"psa{b}", bufs=2, space="PSUM") as psa,
                        tc.tile_pool(name=f"pso{b}", bufs=2, space="PSUM") as pso,
                    ):
                        def emit_C(t, aoT, ics):
                            # out-projection chunks of i-tile t (lagged one
                            # t-step, woven into the next tile's PE stream as
                            # ACT-independent filler work)
                            for ic in ics:
                                ob = osb.tile([128, C], F32, tag="ob", name="ob")
                                for ct in range(C // 512):
                                    op = pso.tile([128, 512], F32, tag="o", name="op")
                                    for h in range(HPC):
                                        nc.tensor.matmul(
                                            op[:],
                                            r(aoT[:, h, (ic - 4 * t) * 128 : (ic - 4 * t) * 128 + 128]),
                                            wo[:, h, ct * 512 : ct * 512 + 512],
                                            start=(h == 0), stop=(h == HPC - 1),
                                        )
                                    nc.vector.tensor_copy(
                                        out=ob[:, ct * 512 : ct * 512 + 512], in_=op[:]
                                    )
                                row0 = b * T + ic * 128
                                nc.sync.dma_start(out_d[row0 : row0 + 128, :], ob[:])

                        pending_C = None
                        for t in reversed(range(NT)):
                            J = 4 * (t + 1)
                            isl = slice(t * 512, t * 512 + 512)
                            aoT = aotp.tile([128, HPC, 512], F32, tag="aot", name="aoT")
                            ebs, dens, aos = {}, {}, {}

                            def rng(jc):
                                return slice(max(128 * jc - 512 * t, 0), 512)

                            for h in range(HPC):
                                if t >= 2:
                                    eb = expb.tile([128, KO, 512], F32, tag="expb", name="eb")
                                else:
                                    eb = exps.tile([128, 8, 512], F32, tag="exps", name="eb")
                                den = psd.tile([128, 512], F32, tag="den", name="den")
                                ao = psa.tile([128, 512], F32, tag="ao", name="ao")
                                ebs[h], dens[h], aos[h] = eb, den, ao

                                def den_pv(jc):
                                    nc.tensor.matmul(
                                        den[:, rng(jc)], ones[:], r(eb[:, jc, rng(jc)]),
                                        start=(jc == 0), stop=(jc == J - 1),
                                        skip_group_check=True,
                                    )
                                    nc.tensor.matmul(
                                        ao[:, rng(jc)],
                                        r(V[:, jc, h * 128 : h * 128 + 128]),
                                        r(eb[:, jc, rng(jc)]),
                                        start=(jc == 0), stop=(jc == J - 1),
                                        skip_group_check=True,
                                    )

                                for jc in range(J):
                                    sp = pss.tile([128, 512], F32, tag="s", name="sp")
                                    nc.tensor.matmul(
                                        sp[:],
                                        r(KT[:, h, jc * 128 : jc * 128 + 128]),
                                        r(QT[:, h, isl]),
                                        start=True, stop=True,
                                    )
                                    bias = cb[:, h * KO + jc : h * KO + jc + 1]
                                    off = 128 * jc - 512 * t
                                    nc.scalar.activation(
                                        r(eb[:, jc, rng(jc)]), sp[:, rng(jc)], EXPF,
                                        bias=bias,
                                    )
                                    if off >= 0:  # causal triangle on diagonal
                                        nc.vector.tensor_mul(
                                            r(eb[:, jc, off : off + 128]),
                                            eb[:, jc, off : off + 128],
                                            tri[:],
                                        )
                                    if jc >= LAG:
                                        den_pv(jc - LAG)
                                for jc in range(max(J - LAG, 0), J):
                                    den_pv(jc)
                                # fast reciprocal (~18 bits, 1 DVE op) fires
                                # right after den completes; normalizes start
                                # during the next head's S^T stream
                                rcp = sbb.tile([128, 512], F32, tag="rcp", name="rcp")
                                nc.vector.reciprocal_approx_fast(out=rcp[:], in_=den[:])
                                nc.vector.tensor_tensor(
                                    r(aoT[:, h, :]), ao[:], rcp[:], MULT
                                )
                                # weave half of the previous i-tile's
                                # out-projection after this head's stream
                                if pending_C is not None:
                                    tp_, aoTp = pending_C
                                    emit_C(tp_, aoTp,
                                           range(4 * tp_ + 2 * h, 4 * tp_ + 2 * h + 2))
                                slot = 2 * h + b
                                for jc in range(J):
                                    st = stp.tile([128, 512], F32, tag="st", name="st")
                                    g = rng(jc)
                                    nc.gpsimd.tensor_mul(
                                        st[:, g], eb[:, jc, g], rcp[:, g]
                                    )
                                    off = max(128 * jc - 512 * t, 0)
                                    nc.sync.dma_start(
                                        attn_r[slot, :, jc, t * 512 + off : (t + 1) * 512],
                                        st[:, g],
                                    )

                            pending_C = (t, aoT)
                        tp_, aoTp = pending_C
                        emit_C(tp_, aoTp, range(4 * tp_, 4 * tp_ + 4))

    nc.compile()
    return nc


def _get_nc():
    if "nc" not in _cached:
        _cached["nc"] = _build()
    return _cached["nc"]


def kernel(x, Wq, Wk, Wv, Wo, bo):
    x = np.asarray(x, np.float32)
    Wq = np.asarray(Wq, np.float32)
    Wk = np.asarray(Wk, np.float32)
    Wv = np.asarray(Wv, np.float32)
    Wo = np.asarray(Wo, np.float32)
    bo = np.asarray(bo, np.float32)

    nc = _get_nc()
    scale = 1.0 / np.sqrt(HD)
    xTr = np.ascontiguousarray(x.reshape(TOK, C).T).reshape(KO, 128, TOK)
    slopes = np.array([1.0 / 2 ** (i + 1) for i in range(H)], np.float32)

    in_maps = []
    for c in range(NCORES):
        rows = slice(256 * c, 256 * c + 256)
        wqT = np.ascontiguousarray((Wq[rows] * scale).T).reshape(KO, 128, 256)
        wkT = np.ascontiguousarray(Wk[rows].T).reshape(KO, 128, 256)
        wvT = np.ascontiguousarray(Wv[rows].T).reshape(KO, 128, 256)
        woT = np.ascontiguousarray(Wo[:, rows].T).reshape(HPC, 128, C)
        p = np.arange(128, dtype=np.float32)[:, None]
        jcs = np.arange(KO, dtype=np.float32)[None, :]
        cbs = []
        for h in range(HPC):
            s = slopes[HPC * c + h]
            cbs.append(-s * (128.0 * jcs + p))  # [128, KO]
        colbias = np.concatenate(cbs, axis=1).astype(np.float32)
        in_maps.append(
            {"xT": xTr, "wqT": wqT, "wkT": wkT, "wvT": wvT, "woT": woT,
             "colbias": colbias}
        )

    res = run_bass_kernel_spmd(nc, in_maps, core_ids=list(range(NCORES)))

    out = np.zeros((TOK, C), np.float32)
    attn = np.empty((B, H, T, T), np.float32)
    for c in range(NCORES):
        out += res.results[c]["outp"]
        at = res.results[c]["attnT"]
        for slot in range(2 * HPC):
            b = slot % 2
            h = HPC * c + slot // 2
            attn[b, h] = at[slot].T
    out = (out + bo[None, :]).reshape(B, T, C)
    return out, attn


if __name__ == "__main__":
    rng = np.random.default_rng(0)
    x = rng.standard_normal((B, T, C)).astype(np.float32)
    Wq = (rng.standard_normal((C, C)) * 0.02).astype(np.float32)
    Wk = (rng.standard_normal((C, C)) * 0.02).astype(np.float32)
    Wv = (rng.standard_normal((C, C)) * 0.02).astype(np.float32)
    Wo = (rng.standard_normal((C, C)) * 0.02).astype(np.float32)
    bo = np.zeros((C,), np.float32)
    out, attn = kernel(x, Wq, Wk, Wv, Wo, bo)
    print("out", out.shape, out.dtype, "attn", attn.shape, attn.dtype)
